# revision 33
# baseline (speedup 1.0000x reference)
"""Trainium2 Bass kernel for the Tucker-factorized (TLE) multi-head attention.

Strategy (v2 — bf16, transpose-free, software-pipelined)
--------------------------------------------------------
Data-parallel over batch: 16 batches / 8 cores = 2 batches per core.

Host-side prep:
 * The three per-mode factor matrices of each projection fold into one dense
   768x768 Kronecker matrix (bf16).  q/k/v rows are permuted to head-major
   order; softmax scale is folded into q; the o matrix columns get the same
   permutation.
 * bk is dropped entirely: softmax is invariant to adding a row-constant
   (q_s . bk) to the scores.
 * bv is folded into bo' = bo + Wo @ bv: softmax rows sum to 1, so the v bias
   shifts y by a constant vector which commutes with the output projection.
 * x is padded to 608 tokens and converted to bf16 (DMA-transpose needs
   16-row tiles and a 2-byte dtype).

Device pipeline, per batch:
  1. x arrives feature-major via 6 hardware DMA transposes (no PE work).
  2. v is projected TOKEN-major (stationary = xT tile, moving = Wv columns),
     landing directly in the [token, head, hd|1] layout PV needs — no
     V transpose.  The trailing ones column yields softmax sums for free.
  3. Per head-pair hp: q/k projected feature-major (stationary = W chunk),
     drained by ScalarE (Identity+bias for q, Copy for k) into bf16 tiles.
     Attention per (hp, g): QK^T into 2-bank PSUM (both 300-token halves),
     one wide exp -> bf16 pp, PV accumulated per half, then a batched
     normalize (fast reciprocal + gpsimd partition broadcast + DVE mul).
     The NEXT head-pair's projection matmuls are interleaved between QK and
     PV so the PE never waits on ScalarE's exp.
  4. Output projection TOKEN-major (stationary = yT tile, moving = Wo
     columns) — no output transpose; DVE adds bo' and converts to bf16;
     DMA straight out.

All matmuls run bf16 (1 cycle/row, half-size LDWEIGHTS); PSUM stays fp32.
Measured end-to-end RMS error ~5e-3 against the fp32 reference (gate 2e-2).
"""

import numpy as np
import ml_dtypes

import concourse.bass as bass
import concourse.tile as tile
from concourse import bacc, mybir
from concourse.bass_utils import run_bass_kernel_spmd

# ---------------------------------------------------------------- constants
N_CORES = 8
B = 16
BPC = B // N_CORES          # batches per core
P1, P2 = 25, 24
S = P1 * P2                 # 600 tokens
SP = 608                    # padded tokens (DMA transpose wants %16 == 0)
F = 768                     # flattened feature dim
FC = 6                      # feature chunks of 128
ST = 120                    # token tile
NS = S // ST                # 5 token tiles
NH = 300                    # half of the token axis
H1, H2, H3 = 2, 2, 3
NHEADS = H1 * H2 * H3       # 12
HD = 64
F32 = mybir.dt.float32
BF16 = mybir.dt.bfloat16
EXP = mybir.ActivationFunctionType.Exp
IDENT = mybir.ActivationFunctionType.Identity
COPYF = mybir.ActivationFunctionType.Copy


# ---------------------------------------------------------------- device IR
def _build_nc():
    nc = bacc.Bacc("TRN2", target_bir_lowering=False, debug=False)
    xr = nc.declare_dram_parameter("x", [BPC, SP, F], BF16, isOutput=False)
    ws = [nc.declare_dram_parameter(f"w{m}", [FC, 128, FC * 128], BF16,
                                    isOutput=False) for m in range(4)]
    bqr = nc.declare_dram_parameter("bq", [F], F32, isOutput=False)
    # bo' replicated host-side to all 128 partitions: a plain DMA is cheaper
    # than a gpsimd partition_broadcast at the head of the gpsimd queue.
    bor = nc.declare_dram_parameter("bo", [128, F], F32, isOutput=False)
    outr = nc.declare_dram_parameter("out", [BPC, S, F], BF16, isOutput=True)

    with tile.TileContext(nc) as tc:
        from contextlib import ExitStack
        with ExitStack() as ctx:
            const = ctx.enter_context(tc.tile_pool(name="const", bufs=1))
            xtp = ctx.enter_context(tc.tile_pool(name="xtp", bufs=2))
            qkp = ctx.enter_context(tc.tile_pool(name="qkp", bufs=4))
            ypl = ctx.enter_context(tc.tile_pool(name="ypl", bufs=2))
            vnp = ctx.enter_context(tc.tile_pool(name="vnp", bufs=2))
            ppp = ctx.enter_context(tc.tile_pool(name="ppp", bufs=4))
            onp = ctx.enter_context(tc.tile_pool(name="onp", bufs=2))
            rcp = ctx.enter_context(tc.tile_pool(name="rcp", bufs=3))
            rbp = ctx.enter_context(tc.tile_pool(name="rbp", bufs=3))
            # PSUM budget 8 banks: pj 2x1 + big 2x2 + py 2x1
            pj = ctx.enter_context(tc.tile_pool(name="pj", bufs=2, space="PSUM"))
            big = ctx.enter_context(tc.tile_pool(name="big", bufs=2, space="PSUM"))
            py = ctx.enter_context(tc.tile_pool(name="py", bufs=2, space="PSUM"))

            # ---- weights: wsb[m] = [128 fi, co, ci, 128 fo] -----------------
            wsb = [const.tile([128, FC, FC, 128], BF16, tag=f"w{m}", name=f"w{m}")
                   for m in range(4)]
            # batch-0 x DMA-transposes go FIRST, split across sync+scalar, so
            # the xbar transfers pipeline before any weight DMA claims the
            # semaphore slots (slot reuse would serialize them).
            # batch-0 x transposes all on the scalar queue: the xbar unit
            # paces them regardless of queue, and this keeps the sync queue
            # free to stream the q/k weights in parallel.
            xT0 = xtp.tile([128, FC, SP], BF16, tag="xT")
            for c in range(FC):
                eng = nc.scalar if c % 2 == 0 else nc.sync
                eng.dma_start_transpose(xT0[:, c, :], xr[0, :, c * 128:(c + 1) * 128])

            # both vn buffers get their ones column exactly once, up front —
            # the v drains only write columns 0:HD, so the ones survive the
            # pool rotation, and the first PV matmul is not gated on a
            # memset stuck behind the weight DMAs.
            vns = [vnp.tile([128, NS, NHEADS, HD + 1], BF16, tag="vn", name=f"vn{i}")
                   for i in range(BPC)]
            for v_ in vns:
                nc.gpsimd.memset(v_[:ST, :, :, HD:HD + 1], 1.0)

            bqs = const.tile([128, FC], F32, tag="bqs")
            nc.scalar.dma_start(out=bqs[:, :], in_=bqr.rearrange("(c p) -> p c", p=128))

            # Consolidated weight DMAs (two halves per matrix): fewer DMAs in
            # flight at startup means far less semaphore-slot recycling,
            # which otherwise llocksteps independent queues. v then bo on
            # gpsimd (the token-major v projection consumes v first; nothing
            # else afterwards so the attention partition_broadcasts aren't
            # stuck behind DMA dispatches); q/k halves then o on sync.
            def w_dma(eng, m, c0, c1):
                eng.dma_start(
                    out=wsb[m][:, c0:c1, :, :],
                    in_=ws[m][c0:c1].rearrange("c p f -> p c f"))

            w_dma(nc.gpsimd, 2, 0, 3)
            w_dma(nc.gpsimd, 2, 3, 6)
            bo_tile = const.tile([128, F], F32, tag="bo_tile")
            nc.gpsimd.dma_start(out=bo_tile[:, :], in_=bor[:, :])
            w_dma(nc.sync, 0, 0, 3)
            w_dma(nc.sync, 1, 0, 3)
            w_dma(nc.sync, 0, 3, 6)
            w_dma(nc.sync, 1, 3, 6)
            w_dma(nc.sync, 3, 0, 3)
            w_dma(nc.sync, 3, 3, 6)

            # prefetch batch-1 x through the (otherwise idle) xbar while
            # batch 0 computes
            xT1 = xtp.tile([128, FC, SP], BF16, tag="xT")
            for c in range(FC):
                nc.sync.dma_start_transpose(xT1[:, c, :], xr[1, :, c * 128:(c + 1) * 128])
            xTs = [xT0, xT1]

            def proj_gen(xT, qT, kT, hp, m):
                """Yield after each accumulation step; drains on close."""
                dst = qT if m == 0 else kT
                acc0 = pj.tile([128, 512], F32, tag="pj")
                acc1 = pj.tile([128, 512], F32, tag="pj")
                for ci in range(FC):
                    st_, sp_ = (ci == 0), (ci == FC - 1)
                    nc.tensor.matmul(acc0[:, 0:NH], wsb[m][:, hp, ci, :],
                                     xT[:, ci, 0:NH], start=st_, stop=sp_)
                    nc.tensor.matmul(acc1[:, 0:NH], wsb[m][:, hp, ci, :],
                                     xT[:, ci, NH:2 * NH], start=st_, stop=sp_)
                    yield
                # drains on DVE: ScalarE is saturated by the attention exps
                if m == 0:
                    nc.vector.tensor_scalar_add(dst[:, hp, 0:NH], in0=acc0[:, 0:NH],
                                                scalar1=bqs[:, hp:hp + 1])
                    nc.vector.tensor_scalar_add(dst[:, hp, NH:2 * NH], in0=acc1[:, 0:NH],
                                                scalar1=bqs[:, hp:hp + 1])
                else:
                    nc.vector.tensor_copy(dst[:, hp, 0:NH], acc0[:, 0:NH])
                    nc.vector.tensor_copy(dst[:, hp, NH:2 * NH], acc1[:, 0:NH])

            def drain_gen(g):
                if g is not None:
                    for _ in g:
                        pass

            for b in range(BPC):
                xT = xTs[b]

                # ---- v projection, token-major -----------------------------
                vn = vns[b]
                for st in range(NS):
                    ps = big.tile([128, 1024], F32, tag="big")
                    xs = xT[:, :, st * ST:(st + 1) * ST]
                    for ci in range(FC):
                        st_, sp_ = (ci == 0), (ci == FC - 1)
                        nc.tensor.matmul(ps[:ST, 0:384], xs[:, ci, :],
                                         wsb[2][:, 0:3, ci, :], start=st_, stop=sp_)
                        nc.tensor.matmul(ps[:ST, 512:896], xs[:, ci, :],
                                         wsb[2][:, 3:6, ci, :], start=st_, stop=sp_)
                    nc.vector.tensor_copy(
                        vn[:ST, st, 0:3 * H1, 0:HD],
                        ps[:ST, 0:384].rearrange("p (h d) -> p h d", d=HD))
                    nc.vector.tensor_copy(
                        vn[:ST, st, 3 * H1:NHEADS, 0:HD],
                        ps[:ST, 512:896].rearrange("p (h d) -> p h d", d=HD))

                # ---- fused per-pair q/k projections + attention ------------
                qT = qkp.tile([128, FC, S], BF16, tag="qkT")
                kT = qkp.tile([128, FC, S], BF16, tag="qkT")
                yT = ypl.tile([128, FC, S], BF16, tag="yT")

                drain_gen(proj_gen(xT, qT, kT, 0, 0))
                drain_gen(proj_gen(xT, qT, kT, 0, 1))

                # deferred normalize: (asb, rb, r0, hp) of the previous
                # stream — its yT multiply is emitted one stream later so the
                # DVE never head-of-line blocks on the gpsimd broadcast.
                pending = [None]

                def flush_pending():
                    if pending[0] is not None:
                        asb_, rb_, r0_, hp_ = pending[0]
                        nc.vector.tensor_mul(
                            yT[r0_:r0_ + HD, hp_, 0:2 * NH].rearrange(
                                "p (a b) -> p a b", a=2),
                            asb_[0:HD, :, :], rb_[:, :, :])
                        pending[0] = None

                for hp in range(FC):
                    for g in range(2):
                        # interleave next head-pair's projection between QK/PV
                        nxt = None
                        if hp + 1 < FC:
                            nxt = proj_gen(xT, qT, kT, hp + 1, g)
                        r0 = g * HD
                        head = hp * 2 + g
                        pp = ppp.tile([128, NS, 2 * NH], BF16, tag="pp")
                        acc0 = py.tile([HD + 1, 512], F32, tag="py")
                        acc1 = py.tile([HD + 1, 512], F32, tag="py")
                        accs = (acc0, acc1)

                        def qk_step(t5):
                            ps = big.tile([128, 1024], F32, tag="big")
                            kst = kT[r0:r0 + HD, hp, t5 * ST:(t5 + 1) * ST]
                            nc.tensor.matmul(ps[:ST, 0:NH], kst,
                                             qT[r0:r0 + HD, hp, 0:NH],
                                             start=True, stop=True)
                            nc.tensor.matmul(ps[:ST, 512:512 + NH], kst,
                                             qT[r0:r0 + HD, hp, NH:2 * NH],
                                             start=True, stop=True)
                            nc.scalar.activation(
                                pp[:ST, t5, :],
                                ps[:ST, :].rearrange("p (h c) -> p h c", h=2)[:, :, 0:NH],
                                func=EXP)

                        def pv_step(t5):
                            st_, sp_ = (t5 == 0), (t5 == NS - 1)
                            for sh in range(2):
                                nc.tensor.matmul(
                                    accs[sh][:HD + 1, 0:NH],
                                    vn[:ST, t5, head, :],
                                    pp[:ST, t5, sh * NH:(sh + 1) * NH],
                                    start=st_, stop=sp_)

                        def pull(n):
                            if nxt is not None:
                                for _ in range(n):
                                    if next(nxt, "done") == "done":
                                        break

                        qk_step(0)
                        for t5 in range(1, NS):
                            qk_step(t5)
                            pull(1)
                            pv_step(t5 - 1)
                        pull(2)
                        pv_step(NS - 1)
                        drain_gen(nxt)

                        # ---- normalize, software-pipelined ----------------
                        # copy accs to SBUF right away (releases the PSUM
                        # banks for the next stream's PV; the fast reciprocal
                        # needs an SBUF, partition-0 source anyway), kick off
                        # the gpsimd broadcast, and only multiply into yT one
                        # stream later.
                        asb = rcp.tile([HD + 1, 2, NH], F32, tag="asb")
                        nc.vector.tensor_copy(asb[:, 0, :], acc0[:HD + 1, 0:NH])
                        nc.vector.tensor_copy(asb[:, 1, :], acc1[:HD + 1, 0:NH])
                        srow = rcp.tile([1, 2, NH], F32, tag="srow")
                        nc.vector.tensor_copy(srow[0:1, :, :], asb[HD:HD + 1, :, :])
                        rec = rcp.tile([1, 2, NH], F32, tag="rec")
                        nc.vector.reciprocal_approx_fast(
                            rec[0:1, :, :], srow[0:1, :, :])
                        rb = rbp.tile([HD, 2, NH], F32, tag="rb")
                        nc.gpsimd.partition_broadcast(rb[:, :, :], rec[0:1, :, :])
                        flush_pending()
                        pending[0] = (asb, rb, r0, hp)

                flush_pending()

                # ---- output projection, token-major ------------------------
                for st in range(NS):
                    ps = big.tile([128, 1024], F32, tag="big")
                    ys = yT[:, :, st * ST:(st + 1) * ST]
                    for ci in range(FC):
                        st_, sp_ = (ci == 0), (ci == FC - 1)
                        nc.tensor.matmul(ps[:ST, 0:384], ys[:, ci, :],
                                         wsb[3][:, 0:3, ci, :], start=st_, stop=sp_)
                        nc.tensor.matmul(ps[:ST, 512:896], ys[:, ci, :],
                                         wsb[3][:, 3:6, ci, :], start=st_, stop=sp_)
                    on = onp.tile([128, F], BF16, tag="on")
                    nc.vector.tensor_add(
                        on[:ST, :].rearrange("p (h c) -> p h c", h=2),
                        ps[:ST, :].rearrange("p (h c) -> p h c", h=2)[:, :, 0:384],
                        bo_tile[:ST, :].rearrange("p (h c) -> p h c", h=2))
                    eng = nc.gpsimd if st % 2 == 0 else nc.sync
                    eng.dma_start(out=outr[b, st * ST:(st + 1) * ST, :],
                                  in_=on[:ST, :])

    nc.finalize()
    return nc


_NC_CACHE = {}


def _get_nc():
    if "nc" not in _NC_CACHE:
        _NC_CACHE["nc"] = _build_nc()
    return _NC_CACHE["nc"]


# ------------------------------------------------------------- host wrapper
def _head_major_perm():
    perm = np.empty(F, dtype=np.int64)
    i = 0
    for h1 in range(H1):
        for h2 in range(H2):
            for h3 in range(H3):
                for x in range(4):
                    for y in range(4):
                        for z in range(4):
                            a = x * H1 + h1
                            bb = y * H2 + h2
                            cc = z * H3 + h3
                            perm[i] = a * 96 + bb * 12 + cc
                            i += 1
    return perm


def _prep_inputs(inputs):
    perm = _head_major_perm()
    scale = float(HD) ** -0.5

    def kron3(w1, w2, w3):
        return np.kron(w1, np.kron(w2, w3)).astype(np.float32)

    def pack(w):
        # [fi, fo] -> dram [co, 128 fi, ci*128 fo]
        w4 = np.ascontiguousarray(
            w.reshape(FC, 128, FC, 128).transpose(2, 1, 0, 3))
        return w4.reshape(FC, 128, FC * 128).astype(ml_dtypes.bfloat16)

    wq = kron3(inputs["Wq1"], inputs["Wq2"], inputs["Wq3"])[perm, :] * scale
    wk = kron3(inputs["Wk1"], inputs["Wk2"], inputs["Wk3"])[perm, :]
    wv = kron3(inputs["Wv1"], inputs["Wv2"], inputs["Wv3"])[perm, :]
    wo = kron3(inputs["Wo1"], inputs["Wo2"], inputs["Wo3"])

    mats = {
        "w0": pack(wq.T),
        "w1": pack(wk.T),
        "w2": pack(wv.T),
        "w3": pack(wo[:, perm].T),
        "bq": np.ascontiguousarray(
            inputs["bq"].reshape(F)[perm] * scale).astype(np.float32),
        # bv folded through the output projection; bk cancels in softmax.
        "bo": np.ascontiguousarray(np.broadcast_to(
            inputs["bo"].reshape(F).astype(np.float32)
            + wo @ inputs["bv"].reshape(F).astype(np.float32), (128, F))),
    }
    return mats


def _make_in_maps(inputs):
    mats = _prep_inputs(inputs)
    x = np.asarray(inputs["x"], dtype=np.float32).reshape(B, S, F)
    xp = np.zeros((B, SP, F), dtype=ml_dtypes.bfloat16)
    xp[:, :S, :] = x.astype(ml_dtypes.bfloat16)
    in_maps = []
    for c in range(N_CORES):
        m = {"x": np.ascontiguousarray(xp[c * BPC:(c + 1) * BPC])}
        m.update(mats)
        in_maps.append(m)
    return in_maps


def _gather(res):
    out = np.concatenate(
        [res.results[c]["out"].astype(np.float32) for c in range(N_CORES)], axis=0)
    return out.reshape(B, P1, P2, 8, 8, 12)


def kernel(**inputs) -> np.ndarray:
    nc = _get_nc()
    in_maps = _make_in_maps(inputs)
    res = run_bass_kernel_spmd(nc, in_maps, core_ids=list(range(N_CORES)))
    return _gather(res)


def run_traced(inputs, **kw):
    """test.py helper: returns (output, BassKernelResults) with trace."""
    nc = _get_nc()
    in_maps = _make_in_maps(inputs)
    res = run_bass_kernel_spmd(nc, in_maps, core_ids=list(range(N_CORES)), **kw)
    return _gather(res), res


# revision 36
# speedup vs baseline: 1.0413x; 1.0413x over previous
"""Trainium2 Bass kernel for the Tucker-factorized (TLE) multi-head attention.

Strategy (v2 — bf16, transpose-free, software-pipelined)
--------------------------------------------------------
Data-parallel over batch: 16 batches / 8 cores = 2 batches per core.

Host-side prep:
 * The three per-mode factor matrices of each projection fold into one dense
   768x768 Kronecker matrix (bf16).  q/k/v rows are permuted to head-major
   order; softmax scale is folded into q; the o matrix columns get the same
   permutation.
 * bk is dropped entirely: softmax is invariant to adding a row-constant
   (q_s . bk) to the scores.
 * bv is folded into bo' = bo + Wo @ bv: softmax rows sum to 1, so the v bias
   shifts y by a constant vector which commutes with the output projection.
 * x is padded to 608 tokens and converted to bf16 (DMA-transpose needs
   16-row tiles and a 2-byte dtype).

Device pipeline, per batch:
  1. x arrives feature-major via 6 hardware DMA transposes (no PE work).
  2. v is projected TOKEN-major (stationary = xT tile, moving = Wv columns),
     landing directly in the [token, head, hd|1] layout PV needs — no
     V transpose.  The trailing ones column yields softmax sums for free.
  3. Per head-pair hp: q/k projected feature-major (stationary = W chunk),
     drained by ScalarE (Identity+bias for q, Copy for k) into bf16 tiles.
     Attention per (hp, g): QK^T into 2-bank PSUM (both 300-token halves),
     one wide exp -> bf16 pp, PV accumulated per half, then a batched
     normalize (fast reciprocal + gpsimd partition broadcast + DVE mul).
     The NEXT head-pair's projection matmuls are interleaved between QK and
     PV so the PE never waits on ScalarE's exp.
  4. Output projection TOKEN-major (stationary = yT tile, moving = Wo
     columns) — no output transpose; DVE adds bo' and converts to bf16;
     DMA straight out.

All matmuls run bf16 (1 cycle/row, half-size LDWEIGHTS); PSUM stays fp32.
Measured end-to-end RMS error ~5e-3 against the fp32 reference (gate 2e-2).
"""

import numpy as np
import ml_dtypes

import concourse.bass as bass
import concourse.tile as tile
from concourse import bacc, mybir
from concourse.bass_utils import run_bass_kernel_spmd

# ---------------------------------------------------------------- constants
N_CORES = 8
B = 16
BPC = B // N_CORES          # batches per core
P1, P2 = 25, 24
S = P1 * P2                 # 600 tokens
SP = 608                    # padded tokens (DMA transpose wants %16 == 0)
F = 768                     # flattened feature dim
FC = 6                      # feature chunks of 128
ST = 120                    # token tile
NS = S // ST                # 5 token tiles
NH = 300                    # half of the token axis
H1, H2, H3 = 2, 2, 3
NHEADS = H1 * H2 * H3       # 12
HD = 64
F32 = mybir.dt.float32
BF16 = mybir.dt.bfloat16
EXP = mybir.ActivationFunctionType.Exp
IDENT = mybir.ActivationFunctionType.Identity
COPYF = mybir.ActivationFunctionType.Copy


# ---------------------------------------------------------------- device IR
def _build_nc():
    nc = bacc.Bacc("TRN2", target_bir_lowering=False, debug=False)
    xr = nc.declare_dram_parameter("x", [BPC, SP, F], BF16, isOutput=False)
    ws = [nc.declare_dram_parameter(f"w{m}", [FC, 128, FC * 128], BF16,
                                    isOutput=False) for m in range(4)]
    bqr = nc.declare_dram_parameter("bq", [F], F32, isOutput=False)
    # bo' replicated host-side to all 128 partitions: a plain DMA is cheaper
    # than a gpsimd partition_broadcast at the head of the gpsimd queue.
    bor = nc.declare_dram_parameter("bo", [128, F], F32, isOutput=False)
    outr = nc.declare_dram_parameter("out", [BPC, S, F], BF16, isOutput=True)

    with tile.TileContext(nc) as tc:
        from contextlib import ExitStack
        with ExitStack() as ctx:
            const = ctx.enter_context(tc.tile_pool(name="const", bufs=1))
            xtp = ctx.enter_context(tc.tile_pool(name="xtp", bufs=2))
            qkp = ctx.enter_context(tc.tile_pool(name="qkp", bufs=4))
            ypl = ctx.enter_context(tc.tile_pool(name="ypl", bufs=2))
            vnp = ctx.enter_context(tc.tile_pool(name="vnp", bufs=2))
            ppp = ctx.enter_context(tc.tile_pool(name="ppp", bufs=4))
            onp = ctx.enter_context(tc.tile_pool(name="onp", bufs=2))
            rcp = ctx.enter_context(tc.tile_pool(name="rcp", bufs=3))
            rbp = ctx.enter_context(tc.tile_pool(name="rbp", bufs=3))
            # PSUM budget 8 banks: pj 2x1 + big 2x2 + py 2x1
            pj = ctx.enter_context(tc.tile_pool(name="pj", bufs=2, space="PSUM"))
            big = ctx.enter_context(tc.tile_pool(name="big", bufs=2, space="PSUM"))
            py = ctx.enter_context(tc.tile_pool(name="py", bufs=2, space="PSUM"))

            # ---- weights: wsb[m] = [128 fi, co, ci, 128 fo] -----------------
            wsb = [const.tile([128, FC, FC, 128], BF16, tag=f"w{m}", name=f"w{m}")
                   for m in range(4)]
            # batch-0 x DMA-transposes go FIRST, split across sync+scalar, so
            # the xbar transfers pipeline before any weight DMA claims the
            # semaphore slots (slot reuse would serialize them).
            # batch-0 x transposes all on the scalar queue: the xbar unit
            # paces them regardless of queue, and this keeps the sync queue
            # free to stream the q/k weights in parallel.
            xT0 = xtp.tile([128, FC, SP], BF16, tag="xT")
            for c in range(FC):
                nc.scalar.dma_start_transpose(xT0[:, c, :], xr[0, :, c * 128:(c + 1) * 128])

            # both vn buffers get their ones column exactly once, up front —
            # the v drains only write columns 0:HD, so the ones survive the
            # pool rotation, and the first PV matmul is not gated on a
            # memset stuck behind the weight DMAs.
            vns = [vnp.tile([128, NS, NHEADS, HD + 1], BF16, tag="vn", name=f"vn{i}")
                   for i in range(BPC)]
            for v_ in vns:
                nc.gpsimd.memset(v_[:ST, :, :, HD:HD + 1], 1.0)

            bqs = const.tile([128, FC], F32, tag="bqs")
            nc.scalar.dma_start(out=bqs[:, :], in_=bqr.rearrange("(c p) -> p c", p=128))

            # Consolidated weight DMAs (two halves per matrix): fewer DMAs in
            # flight at startup means far less semaphore-slot recycling,
            # which otherwise llocksteps independent queues. v then bo on
            # gpsimd (the token-major v projection consumes v first; nothing
            # else afterwards so the attention partition_broadcasts aren't
            # stuck behind DMA dispatches); q/k halves then o on sync.
            def w_dma(eng, m, c0, c1):
                eng.dma_start(
                    out=wsb[m][:, c0:c1, :, :],
                    in_=ws[m][c0:c1].rearrange("c p f -> p c f"))

            w_dma(nc.gpsimd, 2, 0, 3)
            w_dma(nc.gpsimd, 2, 3, 6)
            bo_tile = const.tile([128, F], F32, tag="bo_tile")
            nc.gpsimd.dma_start(out=bo_tile[:, :], in_=bor[:, :])
            w_dma(nc.sync, 0, 0, 3)
            w_dma(nc.sync, 1, 0, 3)
            w_dma(nc.sync, 0, 3, 6)
            w_dma(nc.sync, 1, 3, 6)
            w_dma(nc.sync, 3, 0, 3)
            w_dma(nc.sync, 3, 3, 6)

            # prefetch batch-1 x through the (otherwise idle) xbar while
            # batch 0 computes
            xT1 = xtp.tile([128, FC, SP], BF16, tag="xT")
            for c in range(FC):
                nc.sync.dma_start_transpose(xT1[:, c, :], xr[1, :, c * 128:(c + 1) * 128])
            xTs = [xT0, xT1]

            def proj_gen(xT, qT, kT, hp, m):
                """Yield after each accumulation step; drains on close."""
                dst = qT if m == 0 else kT
                acc0 = pj.tile([128, 512], F32, tag="pj")
                acc1 = pj.tile([128, 512], F32, tag="pj")
                for ci in range(FC):
                    st_, sp_ = (ci == 0), (ci == FC - 1)
                    nc.tensor.matmul(acc0[:, 0:NH], wsb[m][:, hp, ci, :],
                                     xT[:, ci, 0:NH], start=st_, stop=sp_)
                    nc.tensor.matmul(acc1[:, 0:NH], wsb[m][:, hp, ci, :],
                                     xT[:, ci, NH:2 * NH], start=st_, stop=sp_)
                    yield
                # drains on DVE: ScalarE is saturated by the attention exps
                if m == 0:
                    nc.vector.tensor_scalar_add(dst[:, hp, 0:NH], in0=acc0[:, 0:NH],
                                                scalar1=bqs[:, hp:hp + 1])
                    nc.vector.tensor_scalar_add(dst[:, hp, NH:2 * NH], in0=acc1[:, 0:NH],
                                                scalar1=bqs[:, hp:hp + 1])
                else:
                    nc.vector.tensor_copy(dst[:, hp, 0:NH], acc0[:, 0:NH])
                    nc.vector.tensor_copy(dst[:, hp, NH:2 * NH], acc1[:, 0:NH])

            def drain_gen(g):
                if g is not None:
                    for _ in g:
                        pass

            for b in range(BPC):
                xT = xTs[b]

                # ---- v projection, token-major -----------------------------
                vn = vns[b]
                for st in range(NS):
                    ps = big.tile([128, 1024], F32, tag="big")
                    xs = xT[:, :, st * ST:(st + 1) * ST]
                    for ci in range(FC):
                        st_, sp_ = (ci == 0), (ci == FC - 1)
                        nc.tensor.matmul(ps[:ST, 0:384], xs[:, ci, :],
                                         wsb[2][:, 0:3, ci, :], start=st_, stop=sp_)
                        nc.tensor.matmul(ps[:ST, 512:896], xs[:, ci, :],
                                         wsb[2][:, 3:6, ci, :], start=st_, stop=sp_)
                    nc.vector.tensor_copy(
                        vn[:ST, st, 0:3 * H1, 0:HD],
                        ps[:ST, 0:384].rearrange("p (h d) -> p h d", d=HD))
                    nc.vector.tensor_copy(
                        vn[:ST, st, 3 * H1:NHEADS, 0:HD],
                        ps[:ST, 512:896].rearrange("p (h d) -> p h d", d=HD))

                # ---- fused per-pair q/k projections + attention ------------
                qT = qkp.tile([128, FC, S], BF16, tag="qkT")
                kT = qkp.tile([128, FC, S], BF16, tag="qkT")
                yT = ypl.tile([128, FC, S], BF16, tag="yT")

                drain_gen(proj_gen(xT, qT, kT, 0, 0))
                drain_gen(proj_gen(xT, qT, kT, 0, 1))

                # deferred normalize: (asb, rb, r0, hp) of the previous
                # stream — its yT multiply is emitted one stream later so the
                # DVE never head-of-line blocks on the gpsimd broadcast.
                pending = [None]

                def flush_pending():
                    if pending[0] is not None:
                        asb_, rb_, r0_, hp_ = pending[0]
                        nc.vector.tensor_mul(
                            yT[r0_:r0_ + HD, hp_, 0:2 * NH].rearrange(
                                "p (a b) -> p a b", a=2),
                            asb_[0:HD, :, :], rb_[:, :, :])
                        pending[0] = None

                for hp in range(FC):
                    for g in range(2):
                        # interleave next head-pair's projection between QK/PV
                        nxt = None
                        if hp + 1 < FC:
                            nxt = proj_gen(xT, qT, kT, hp + 1, g)
                        r0 = g * HD
                        head = hp * 2 + g
                        pp = ppp.tile([128, NS, 2 * NH], BF16, tag="pp")
                        acc0 = py.tile([HD + 1, 512], F32, tag="py")
                        acc1 = py.tile([HD + 1, 512], F32, tag="py")
                        accs = (acc0, acc1)

                        def qk_step(t5):
                            ps = big.tile([128, 1024], F32, tag="big")
                            kst = kT[r0:r0 + HD, hp, t5 * ST:(t5 + 1) * ST]
                            nc.tensor.matmul(ps[:ST, 0:NH], kst,
                                             qT[r0:r0 + HD, hp, 0:NH],
                                             start=True, stop=True)
                            nc.tensor.matmul(ps[:ST, 512:512 + NH], kst,
                                             qT[r0:r0 + HD, hp, NH:2 * NH],
                                             start=True, stop=True)
                            nc.scalar.activation(
                                pp[:ST, t5, :],
                                ps[:ST, :].rearrange("p (h c) -> p h c", h=2)[:, :, 0:NH],
                                func=EXP)

                        def pv_step(t5):
                            st_, sp_ = (t5 == 0), (t5 == NS - 1)
                            for sh in range(2):
                                nc.tensor.matmul(
                                    accs[sh][:HD + 1, 0:NH],
                                    vn[:ST, t5, head, :],
                                    pp[:ST, t5, sh * NH:(sh + 1) * NH],
                                    start=st_, stop=sp_)

                        def pull(n):
                            if nxt is not None:
                                for _ in range(n):
                                    if next(nxt, "done") == "done":
                                        break

                        qk_step(0)
                        for t5 in range(1, NS):
                            qk_step(t5)
                            pull(1)
                            pv_step(t5 - 1)
                        pull(2)
                        pv_step(NS - 1)
                        drain_gen(nxt)

                        # ---- normalize, software-pipelined ----------------
                        # copy accs to SBUF right away (releases the PSUM
                        # banks for the next stream's PV; the fast reciprocal
                        # needs an SBUF, partition-0 source anyway), kick off
                        # the gpsimd broadcast, and only multiply into yT one
                        # stream later.
                        asb = rcp.tile([HD + 1, 2, NH], F32, tag="asb")
                        nc.vector.tensor_copy(asb[:, 0, :], acc0[:HD + 1, 0:NH])
                        nc.vector.tensor_copy(asb[:, 1, :], acc1[:HD + 1, 0:NH])
                        srow = rcp.tile([1, 2, NH], F32, tag="srow")
                        nc.vector.tensor_copy(srow[0:1, :, :], asb[HD:HD + 1, :, :])
                        rec = rcp.tile([1, 2, NH], F32, tag="rec")
                        nc.vector.reciprocal_approx_fast(
                            rec[0:1, :, :], srow[0:1, :, :])
                        rb = rbp.tile([HD, 2, NH], F32, tag="rb")
                        nc.gpsimd.partition_broadcast(rb[:, :, :], rec[0:1, :, :])
                        flush_pending()
                        pending[0] = (asb, rb, r0, hp)

                flush_pending()

                # ---- output projection, token-major ------------------------
                for st in range(NS):
                    ps = big.tile([128, 1024], F32, tag="big")
                    ys = yT[:, :, st * ST:(st + 1) * ST]
                    for ci in range(FC):
                        st_, sp_ = (ci == 0), (ci == FC - 1)
                        nc.tensor.matmul(ps[:ST, 0:384], ys[:, ci, :],
                                         wsb[3][:, 0:3, ci, :], start=st_, stop=sp_)
                        nc.tensor.matmul(ps[:ST, 512:896], ys[:, ci, :],
                                         wsb[3][:, 3:6, ci, :], start=st_, stop=sp_)
                    on = onp.tile([128, F], BF16, tag="on")
                    nc.vector.tensor_add(
                        on[:ST, :].rearrange("p (h c) -> p h c", h=2),
                        ps[:ST, :].rearrange("p (h c) -> p h c", h=2)[:, :, 0:384],
                        bo_tile[:ST, :].rearrange("p (h c) -> p h c", h=2))
                    eng = nc.gpsimd if st % 2 == 0 else nc.sync
                    eng.dma_start(out=outr[b, st * ST:(st + 1) * ST, :],
                                  in_=on[:ST, :])

    nc.finalize()
    return nc


_NC_CACHE = {}


def _get_nc():
    if "nc" not in _NC_CACHE:
        _NC_CACHE["nc"] = _build_nc()
    return _NC_CACHE["nc"]


# ------------------------------------------------------------- host wrapper
def _head_major_perm():
    perm = np.empty(F, dtype=np.int64)
    i = 0
    for h1 in range(H1):
        for h2 in range(H2):
            for h3 in range(H3):
                for x in range(4):
                    for y in range(4):
                        for z in range(4):
                            a = x * H1 + h1
                            bb = y * H2 + h2
                            cc = z * H3 + h3
                            perm[i] = a * 96 + bb * 12 + cc
                            i += 1
    return perm


def _prep_inputs(inputs):
    perm = _head_major_perm()
    scale = float(HD) ** -0.5

    def kron3(w1, w2, w3):
        return np.kron(w1, np.kron(w2, w3)).astype(np.float32)

    def pack(w):
        # [fi, fo] -> dram [co, 128 fi, ci*128 fo]
        w4 = np.ascontiguousarray(
            w.reshape(FC, 128, FC, 128).transpose(2, 1, 0, 3))
        return w4.reshape(FC, 128, FC * 128).astype(ml_dtypes.bfloat16)

    wq = kron3(inputs["Wq1"], inputs["Wq2"], inputs["Wq3"])[perm, :] * scale
    wk = kron3(inputs["Wk1"], inputs["Wk2"], inputs["Wk3"])[perm, :]
    wv = kron3(inputs["Wv1"], inputs["Wv2"], inputs["Wv3"])[perm, :]
    wo = kron3(inputs["Wo1"], inputs["Wo2"], inputs["Wo3"])

    mats = {
        "w0": pack(wq.T),
        "w1": pack(wk.T),
        "w2": pack(wv.T),
        "w3": pack(wo[:, perm].T),
        "bq": np.ascontiguousarray(
            inputs["bq"].reshape(F)[perm] * scale).astype(np.float32),
        # bv folded through the output projection; bk cancels in softmax.
        "bo": np.ascontiguousarray(np.broadcast_to(
            inputs["bo"].reshape(F).astype(np.float32)
            + wo @ inputs["bv"].reshape(F).astype(np.float32), (128, F))),
    }
    return mats


def _make_in_maps(inputs):
    mats = _prep_inputs(inputs)
    x = np.asarray(inputs["x"], dtype=np.float32).reshape(B, S, F)
    xp = np.zeros((B, SP, F), dtype=ml_dtypes.bfloat16)
    xp[:, :S, :] = x.astype(ml_dtypes.bfloat16)
    in_maps = []
    for c in range(N_CORES):
        m = {"x": np.ascontiguousarray(xp[c * BPC:(c + 1) * BPC])}
        m.update(mats)
        in_maps.append(m)
    return in_maps


def _gather(res):
    out = np.concatenate(
        [res.results[c]["out"].astype(np.float32) for c in range(N_CORES)], axis=0)
    return out.reshape(B, P1, P2, 8, 8, 12)


def kernel(**inputs) -> np.ndarray:
    nc = _get_nc()
    in_maps = _make_in_maps(inputs)
    res = run_bass_kernel_spmd(nc, in_maps, core_ids=list(range(N_CORES)))
    return _gather(res)


def run_traced(inputs, **kw):
    """test.py helper: returns (output, BassKernelResults) with trace."""
    nc = _get_nc()
    in_maps = _make_in_maps(inputs)
    res = run_bass_kernel_spmd(nc, in_maps, core_ids=list(range(N_CORES)), **kw)
    return _gather(res), res
